# revision 20
# baseline (speedup 1.0000x reference)
"""TRN2 Bass kernel for nn_Attention_59270548685139.

Custom two-stage-normalized attention, B=8, N=1024, D=1024, H=8, DH=64.
Sharding: data-parallel over batch -- one batch element per NeuronCore (8 cores).

Math per batch element (matching the reference):
  q = x @ Wq, k = x @ Wk, v = x @ Wv          (split into 8 heads of 64)
  sim[i,j]  = (q_i . k_j) * DH**-0.5
  attn      = softmax over the QUERY dim i    -> E[i,j]/C[j], C[j] = sum_i E[i,j]
  attn      = attn / (sum_j attn + eps)       -> per-i scale 1/(R[i]+eps)
  out       = attn @ v ; y = out @ Wo + bo

Layout: scores are computed transposed (S^T[j,i]) so the softmax-over-queries
reduction is fused into the ACT exp pass (accum_out -> C[j]). The key-dim
renormalization folds into a per-partition scale of V (1/C[j], via GPSIMD
normalize_recip) with an appended 1/C column so the attn@v matmul also
produces R[i]. All matmuls are fp32r (full PE rate, ~1e-4 precision).

Per-head normalization tail (1/R -> broadcast -> multiply) is software-
pipelined TWO heads deep so the slow DVE reciprocal (~6.5us) never
head-of-line-blocks any engine queue: during head h we run reciprocal(h-1)
and broadcast+multiply(h-2), all of whose inputs are already available.
The attn@v accumulators are drained PSUM->SBUF immediately so PSUM stays at
8 banks: 3x[128,1024] score/projection tiles + 2x[65,512] attn@v.
E-tiles alias the x^T tiles (same pool tags) -- x^T dies when attention
starts, freeing the space, which lets all four weight matrices be resident
from t=0 (no mid-kernel weight-DMA stalls).
"""

import numpy as np

import concourse.bass as bass
import concourse.tile as tile
from concourse import bacc, mybir
from concourse.bass_utils import run_bass_kernel_spmd
from concourse.masks import make_identity

FP32 = mybir.dt.float32
FP32R = mybir.dt.float32r

B, N, D = 8, 1024, 1024
H, DH = 8, 64
INNER = H * DH  # 512
SCALE = DH ** -0.5
EPS = 1e-7
P = 128
NCORES = 8

_NC_CACHE = None


def _build_nc():
    nc = bacc.Bacc("TRN2", target_bir_lowering=False, debug=False)

    x_d = nc.dram_tensor("x", [N, D], FP32, kind="ExternalInput")
    wq_d = nc.dram_tensor("Wq", [D, INNER], FP32, kind="ExternalInput")
    wk_d = nc.dram_tensor("Wk", [D, INNER], FP32, kind="ExternalInput")
    wv_d = nc.dram_tensor("Wv", [D, INNER], FP32, kind="ExternalInput")
    wo_d = nc.dram_tensor("Wo", [INNER, D], FP32, kind="ExternalInput")
    bo_d = nc.dram_tensor("bo", [D], FP32, kind="ExternalInput")
    y_d = nc.dram_tensor("y", [N, D], FP32, kind="ExternalOutput")

    DC = D // P       # 8 contraction chunks over D
    IC = INNER // P   # 4 chunks over INNER
    NB = N // P       # 8 seq blocks of 128

    with tile.TileContext(nc) as tc:
        # ---------------- pools (all persistent; no phase barriers) ---------
        const_pool = tc.alloc_tile_pool(name="const", bufs=1)
        qt_pool = tc.alloc_tile_pool(name="qt", bufs=1)
        kt_pool = tc.alloc_tile_pool(name="kt", bufs=1)
        v_pool = tc.alloc_tile_pool(name="v", bufs=1)
        ot_pool = tc.alloc_tile_pool(name="ot", bufs=1)
        xt_pool = tc.alloc_tile_pool(name="xt", bufs=1)
        w3_pool = tc.alloc_tile_pool(name="w3", bufs=4)
        xn_pool = tc.alloc_tile_pool(name="xn", bufs=2)
        sm_pool = tc.alloc_tile_pool(name="sm", bufs=2)
        smb_pool = tc.alloc_tile_pool(name="smb", bufs=2)
        usb_pool = tc.alloc_tile_pool(name="usb", bufs=3)
        y_pool = tc.alloc_tile_pool(name="yp", bufs=2)
        ps_pool = tc.alloc_tile_pool(name="ps", bufs=2, space="PSUM")

        # ---------------- constants ----------------
        ident = const_pool.tile([P, P], FP32, tag="ident")
        make_identity(nc, ident[:])
        ones_f = const_pool.tile([1, P], FP32, tag="ones_f")
        nc.vector.memset(ones_f[:], 1.0)
        ones_r = const_pool.tile([1, P], FP32R, tag="ones_r")
        nc.vector.tensor_copy(ones_r[:], ones_f[:])
        # bo as [1, 2, 512] fp32r (free-dim block db = bo[db*512:(db+1)*512])
        bo_r = const_pool.tile([1, 2, 512], FP32R, tag="bo_r")
        nc.sync.dma_start(
            out=bo_r[:],
            in_=bo_d.ap().rearrange("(a n) -> a n", a=2)[None, :, :].bitcast(FP32R),
        )

        # ---------------- persistent intermediates ----------------
        qt = [qt_pool.tile([P, N], FP32R, tag=f"qt{m}", name=f"qt{m}") for m in range(IC)]
        kt = [kt_pool.tile([P, N], FP32R, tag=f"kt{m}", name=f"kt{m}") for m in range(IC)]
        vts = [v_pool.tile([P, INNER], FP32, tag=f"v{j}", name=f"v{j}") for j in range(NB)]
        ot = [ot_pool.tile([P, N], FP32R, tag=f"ot{m}", name=f"ot{m}") for m in range(IC)]
        xt = [xt_pool.tile([P, N], FP32R, tag=f"xt{c}", name=f"xt{c}") for c in range(DC)]

        # ---------------- phase A: load x, transpose to xt ----------------
        w_ts = {}

        def _load_w(key, wd):
            w_ts[key] = w3_pool.tile([P, DC, INNER], FP32R, tag="w", name=f"w_{key}")
            nc.sync.dma_start(
                out=w_ts[key][:],
                in_=wd.ap().rearrange("(c p) n -> p c n", p=P).bitcast(FP32R),
            )

        for ib in range(NB):
            if ib == 4:
                _load_w("q", wq_d)
                _load_w("k", wk_d)
            xn_t = xn_pool.tile([P, D], FP32, tag="xn", name=f"xn{ib}")
            nc.sync.dma_start(out=xn_t[:], in_=x_d.ap()[ib * P:(ib + 1) * P, :])
            p_t = ps_pool.tile([P, N], FP32, tag="big", name=f"ptp{ib}", bufs=3)
            for c in range(DC):
                nc.tensor.transpose(
                    p_t[:, c * P:(c + 1) * P],
                    xn_t[:, c * P:(c + 1) * P],
                    ident[:],
                )
            for c in range(DC):
                if c % 2 == 0:
                    nc.scalar.copy(
                        xt[c][:, ib * P:(ib + 1) * P], p_t[:, c * P:(c + 1) * P]
                    )
                else:
                    nc.vector.tensor_copy(
                        xt[c][:, ib * P:(ib + 1) * P], p_t[:, c * P:(c + 1) * P]
                    )

        _load_w("v", wv_d)
        wo_t = w3_pool.tile([P, IC, D], FP32R, tag="w", name="w_o")
        nc.sync.dma_start(
            out=wo_t[:],
            in_=wo_d.ap().rearrange("(c p) n -> p c n", p=P).bitcast(FP32R),
        )

        # ---------------- phase B: projections ----------------
        # QT/KT: [INNER, N] head-major; chunk mb = head pair (2mb, 2mb+1)
        for mb in range(IC):
            for key, dst in (("q", qt), ("k", kt)):
                p_t = ps_pool.tile([P, N], FP32, tag="big", name=f"pp{key}_{mb}", bufs=3)
                for ih in range(2):
                    for c in range(DC):
                        nc.tensor.matmul(
                            p_t[:, ih * 512:(ih + 1) * 512],
                            w_ts[key][:, c, mb * P:(mb + 1) * P],
                            xt[c][:, ih * 512:(ih + 1) * 512],
                            start=(c == 0), stop=(c == DC - 1),
                        )
                nc.vector.tensor_copy(dst[mb][:], p_t[:])

        # V natural: [N, INNER]
        for jp in range(4):
            p_t = ps_pool.tile([P, N], FP32, tag="big", name=f"pv{jp}", bufs=3)
            for half in range(2):
                jb = 2 * jp + half
                for c in range(DC):
                    nc.tensor.matmul(
                        p_t[:, half * 512:(half + 1) * 512],
                        xt[c][:, jb * P:(jb + 1) * P],
                        w_ts["v"][:, c, :],
                        start=(c == 0), stop=(c == DC - 1),
                    )
                nc.scalar.copy(vts[jb][:], p_t[:, half * 512:(half + 1) * 512])

        # ---------------- phase C: attention per head ----------------
        # Two-deep software pipeline for the normalization tail:
        #   during head h: reciprocal for head h-1, broadcast+multiply for h-2.
        us_tiles = {}
        rrec_tiles = {}

        def emit_recip(g):
            rrec = smb_pool.tile([1, N], FP32, tag="rrec", name=f"rrec{g}")
            nc.vector.reciprocal(rrec[:], us_tiles[g][DH:DH + 1, :])
            rrec_tiles[g] = rrec

        def emit_finish(g):
            gmb, goff = g // 2, (g % 2) * DH
            bc_sb = sm_pool.tile([DH, N], FP32, tag="bc_sb", name=f"bcs{g}")
            nc.gpsimd.partition_broadcast(bc_sb[:], rrec_tiles[g][:])
            nc.vector.tensor_mul(
                ot[gmb][goff:goff + DH, :],
                us_tiles[g][0:DH, :],
                bc_sb[:],
            )

        for h in range(H):
            mb, off = h // 2, (h % 2) * DH
            kth = kt[mb][off:off + DH, :]
            qth = qt[mb][off:off + DH, :]

            if h >= 2:
                emit_finish(h - 2)
            if h >= 1:
                emit_recip(h - 1)

            c_all = sm_pool.tile([P, NB], FP32, tag="c_all", name=f"ca{h}")
            v2all = sm_pool.tile([P, NB, DH + 1], FP32R, tag="v2", name=f"v2_{h}", bufs=1)
            ets = []
            for jb in range(NB):
                # S^T block [128 j, 1024 i] in PSUM (2 banks)
                p_s = ps_pool.tile([P, N], FP32, tag="big", name=f"s{h}_{jb}", bufs=3)
                for ih in range(2):
                    nc.tensor.matmul(
                        p_s[:, ih * 512:(ih + 1) * 512],
                        kth[:, jb * P:(jb + 1) * P],
                        qth[:, ih * 512:(ih + 1) * 512],
                        start=True, stop=True,
                    )
                # fused exp + softmax-denominator C[j]; rounds to fp32r
                # (E aliases the x^T tiles: same tag)
                et = xt_pool.tile([P, N], FP32R, tag=f"xt{jb}", name=f"et{h}_{jb}")
                nc.scalar.activation(
                    et[:], p_s[:], mybir.ActivationFunctionType.Exp,
                    scale=SCALE, accum_out=c_all[:, jb:jb + 1],
                )
                ets.append(et)
                # V' = V / C[j] on GPSIMD; c_all[:, jb] becomes 1/C in place
                nc.gpsimd.normalize_recip(
                    v2all[:, jb, 0:DH],
                    vts[jb][:, h * DH:(h + 1) * DH],
                    c_all[:, jb:jb + 1],
                )
                nc.gpsimd.tensor_copy(v2all[:, jb, DH:DH + 1], c_all[:, jb:jb + 1])

            # U^T[d, i] accumulated over j ; row DH = R[i]
            p_us = []
            for ih in range(2):
                p_u = ps_pool.tile([DH + 1, 512], FP32, tag="u", name=f"u{h}_{ih}", bufs=2)
                for jb in range(NB):
                    nc.tensor.matmul(
                        p_u[:],
                        v2all[:, jb, :],
                        ets[jb][:, ih * 512:(ih + 1) * 512],
                        start=(jb == 0), stop=(jb == NB - 1),
                    )
                p_us.append(p_u)

            # drain U to SBUF right away (frees PSUM, decouples the tail)
            us = usb_pool.tile([DH + 1, N], FP32, tag="usb", name=f"usb{h}")
            for ih in range(2):
                nc.vector.tensor_copy(
                    us[:, ih * 512:(ih + 1) * 512], p_us[ih][:]
                )
            us_tiles[h] = us

        emit_finish(H - 2)
        emit_recip(H - 1)
        emit_finish(H - 1)

        # ---------------- phase D: output projection (+bo via K=1 matmul) ----
        for ib in range(NB):
            p_y = ps_pool.tile([P, N], FP32, tag="big", name=f"py{ib}", bufs=3)
            for db in range(2):
                nc.tensor.matmul(
                    p_y[:, db * 512:(db + 1) * 512],
                    ones_r[:], bo_r[:, db, :],
                    start=True, stop=False,
                )
                for mbi in range(IC):
                    nc.tensor.matmul(
                        p_y[:, db * 512:(db + 1) * 512],
                        ot[mbi][:, ib * P:(ib + 1) * P],
                        wo_t[:, mbi, db * 512:(db + 1) * 512],
                        start=False, stop=(mbi == IC - 1),
                    )
            for db in range(2):
                y_t = y_pool.tile([P, 512], FP32, tag="y", name=f"y{ib}_{db}")
                nc.vector.tensor_copy(y_t[:], p_y[:, db * 512:(db + 1) * 512])
                nc.sync.dma_start(
                    out=y_d.ap()[ib * P:(ib + 1) * P, db * 512:(db + 1) * 512],
                    in_=y_t[:],
                )

        for p in (ps_pool, y_pool, usb_pool, smb_pool, sm_pool, xn_pool,
                  w3_pool, xt_pool, ot_pool, v_pool, kt_pool, qt_pool,
                  const_pool):
            p.release()

    nc.finalize()
    return nc


def _get_nc():
    global _NC_CACHE
    if _NC_CACHE is None:
        _NC_CACHE = _build_nc()
    return _NC_CACHE


def kernel(x, Wq, Wk, Wv, Wo, bo, _trace=False, **trace_kwargs):
    x = np.ascontiguousarray(np.asarray(x, dtype=np.float32))
    Wq = np.ascontiguousarray(np.asarray(Wq, dtype=np.float32))
    Wk = np.ascontiguousarray(np.asarray(Wk, dtype=np.float32))
    Wv = np.ascontiguousarray(np.asarray(Wv, dtype=np.float32))
    Wo = np.ascontiguousarray(np.asarray(Wo, dtype=np.float32))
    bo = np.ascontiguousarray(np.asarray(bo, dtype=np.float32))

    nc = _get_nc()
    in_maps = [
        {"x": x[c], "Wq": Wq, "Wk": Wk, "Wv": Wv, "Wo": Wo, "bo": bo}
        for c in range(NCORES)
    ]
    res = run_bass_kernel_spmd(
        nc, in_maps, core_ids=list(range(NCORES)), trace=_trace, **trace_kwargs
    )
    out = np.stack([res.results[c]["y"] for c in range(NCORES)], axis=0)
    if _trace:
        return out.astype(np.float32), res
    return out.astype(np.float32)


if __name__ == "__main__":
    rng = np.random.default_rng(0)
    xs = rng.standard_normal((B, N, D), dtype=np.float32)
    wq = rng.standard_normal((D, INNER), dtype=np.float32) * D ** -0.5
    wk = rng.standard_normal((D, INNER), dtype=np.float32) * D ** -0.5
    wv = rng.standard_normal((D, INNER), dtype=np.float32) * D ** -0.5
    wo = rng.standard_normal((INNER, D), dtype=np.float32) * INNER ** -0.5
    bz = np.zeros((D,), dtype=np.float32)
    y = kernel(xs, wq, wk, wv, wo, bz)
    print("ran ok", y.shape, float(np.abs(y).mean()))
